# revision 13
# baseline (speedup 1.0000x reference)
"""Trainium2 Bass kernel for nn_DecoupledPointJAFAR (gnn_message_passing).

Strategy (self-contained; hardcoded for B=4, N=8192, K=16, QK=64, NC=13, CIN=6):
  - 8 NeuronCores, data-parallel: core c handles batch b=c//2, query half
    h=c%2 (4096 query points). Each core computes the pointwise encoder for
    ALL 8192 points of its batch element (cheap), then the KNN attention for
    its own 4096 queries.
  - Point order is remapped per-core so the core's own query half is always
    local rows [0, 4096) — keeps the program SPMD (identical on all cores);
    k_idx values are remapped on the host to match.
  - Host-folded algebra:
      attn_logits[n,k] = (Q'[n]·geom[i] + Q''[n]·relu(p[n] - v[i]))/8 + c(n)
    with Q' = (k_w^T q_w)@geom + k_w^T q_b, Q'' = (rp2_w^T q_w)@geom + ...,
    v[i] = (diag(s_rp)·rp1_w)@xyz[i], p[n] = v[n] + (s_rp*rp1_b + t_rp).
    c(n) is constant over k and cancels in softmax; /8 is folded into Q',Q''.
    BN layers fold into per-channel scale+bias applied by ACT on PSUM drains.
  - A DRAM table G holds 512B rows [geom_f32(64) | v_bf16(64) | geom_bf16(64)];
    one dma_gather per query chunk fetches neighbor rows. The pre-softmax
    pipeline runs in bf16; softmax and out_feat accumulate in fp32.
"""

import os
import sys
from contextlib import ExitStack

import numpy as np

for _p in ("/opt/trn_rl_repo", "/opt/pypackages"):
    if _p not in sys.path and os.path.isdir(_p):
        sys.path.append(_p)

import concourse.bass as bass
import concourse.mybir as mybir
import concourse.tile as tile
from concourse import bacc
from concourse.bass_utils import run_bass_kernel_spmd

F32 = mybir.dt.float32
BF16 = mybir.dt.bfloat16
I16 = mybir.dt.int16
AF = mybir.ActivationFunctionType
OP = mybir.AluOpType
AX = mybir.AxisListType

B, N, K, QK, NCLS, CIN = 4, 8192, 16, 64, 13, 6
EPS = 1e-5
NQ = N // 2          # queries per core
NCORES = 8
CH = 16              # gather chunks per core
QC = NQ // CH        # 256 queries per chunk
TPC = QC // 128      # 2 query-tiles per chunk
NB = NQ // 128       # 32 query-tiles per core
IPC = QC * K         # 4096 indices per chunk
GROW = 128           # G row: 128 f32 = 512 B
NC1 = NQ // 512      # 8 column-chunks for stage 1

_CACHE: dict = {}


def _bn_fold(bnp):
    s = np.asarray(bnp['gamma'], np.float64) / np.sqrt(np.asarray(bnp['var'], np.float64) + EPS)
    t = np.asarray(bnp['beta'], np.float64) - np.asarray(bnp['mean'], np.float64) * s
    return s.astype(np.float32), t.astype(np.float32)


def _blkdiag2(w):
    a, b = w.shape
    out = np.zeros((2 * a, 2 * b), np.float32)
    out[:a, :b] = w
    out[a:, b:] = w
    return out


def _host_weights(params):
    p = {k: ({kk: np.asarray(vv, np.float32) for kk, vv in v.items()}
             if isinstance(v, dict) else np.asarray(v, np.float32))
         for k, v in params.items()}
    s1, t1 = _bn_fold(p['ge_bn1'])
    s2, t2 = _bn_fold(p['ge_bn2'])
    sb, tb = _bn_fold(p['bdy_bn'])
    sr, tr = _bn_fold(p['rp_bn'])

    A1 = (p['k_w'].T @ p['q_w']) / 8.0
    c1 = (p['k_w'].T @ p['q_b']) / 8.0
    A2 = (p['rp2_w'].T @ p['q_w']) / 8.0
    c2 = (p['rp2_w'].T @ p['q_b']) / 8.0
    R = sr[:, None] * p['rp1_w']
    wrp = sr * p['rp1_b'] + tr

    W = {}
    W['L1'] = _blkdiag2(p['ge1_w'].T)
    W['L2'] = _blkdiag2(p['ge2_w'].T)
    W['L3a'] = _blkdiag2(p['scale_w'].T)
    W['L3b'] = _blkdiag2(p['shift_w'].T)
    W['L4'] = _blkdiag2(R.T)
    W['L5'] = np.concatenate([A1.T, A2.T], axis=1).astype(np.float32)
    W['L6'] = p['bdy1_w'].T.copy()
    W['L7'] = p['bdy2_w'].T.copy()
    W['L8'] = p['cls_w'].T.copy()

    def st2(x):
        return np.concatenate([x, x]).astype(np.float32)

    bias = np.zeros((128, 12), np.float32)
    bias[:, 0] = st2(s1)
    bias[:, 1] = st2(s1 * p['ge1_b'] + t1)
    bias[:, 2] = st2(s2)
    bias[:, 3] = st2(s2 * p['ge2_b'] + t2)
    bias[:, 4] = st2(p['scale_b'] + 1.0)
    bias[:, 5] = st2(p['shift_b'])
    bias[:, 6] = st2(wrp)
    bias[:, 7] = np.concatenate([c1, c2]).astype(np.float32)
    bias[:32, 8] = sb
    bias[:32, 9] = sb * p['bdy1_b'] + tb
    bias[0, 10] = p['bdy2_b'][0]
    bias[:NCLS, 11] = p['cls_b']
    W['bias'] = bias
    W['ident'] = np.eye(128, dtype=np.float32)
    return W


def _host_prep(xyz, feat, k_idx, params):
    W = _host_weights(params)
    in_maps = []
    for c in range(NCORES):
        b, h = c // 2, c % 2
        if h == 0:
            perm_feat, perm_xyz = feat[b], xyz[b]
        else:
            perm_feat = np.concatenate([feat[b, NQ:], feat[b, :NQ]], axis=0)
            perm_xyz = np.concatenate([xyz[b, NQ:], xyz[b, :NQ]], axis=0)
        featT = np.ascontiguousarray(perm_feat.reshape(2, NQ, CIN)
                                     .transpose(0, 2, 1).reshape(2 * CIN, NQ))
        xyzT = np.ascontiguousarray(perm_xyz.reshape(2, NQ, 3)
                                    .transpose(0, 2, 1).reshape(6, NQ))
        idx_loc = (k_idx[b, h * NQ:(h + 1) * NQ].astype(np.int64) - h * NQ) % N
        idx16 = np.zeros((128, CH * QC), np.int16)
        for ci in range(CH):
            blk = idx_loc[ci * QC:(ci + 1) * QC]           # (QC, K)
            flat = blk.T.reshape(-1)                        # k-major: j = k*QC + q
            wrapped = flat.reshape(IPC // 16, 16).T         # (16, QC)
            idx16[:, ci * QC:(ci + 1) * QC] = np.tile(wrapped, (8, 1)).astype(np.int16)
        m = dict(W)
        m['featT'] = featT
        m['xyzT'] = xyzT
        m['idx'] = idx16
        in_maps.append(m)
    return in_maps


def _build_nc():
    nc = bacc.Bacc("TRN2", target_bir_lowering=False, debug=False)

    di = {}
    def din(name, shape, dt=F32):
        di[name] = nc.dram_tensor(name, list(shape), dt, kind="ExternalInput").ap()

    din('featT', (2 * CIN, NQ))
    din('xyzT', (6, NQ))
    din('idx', (128, CH * QC), I16)
    din('L1', (12, 128)); din('L2', (128, 128)); din('L3a', (12, 128))
    din('L3b', (12, 128)); din('L4', (6, 128)); din('L5', (64, 128))
    din('L6', (64, 32)); din('L7', (32, 1)); din('L8', (64, NCLS))
    din('bias', (128, 12)); din('ident', (128, 128))

    do = {}
    do['aff_o'] = nc.dram_tensor('aff_o', [NQ, K], F32, kind="ExternalOutput").ap()
    do['of_o'] = nc.dram_tensor('of_o', [NQ, QK], F32, kind="ExternalOutput").ap()
    do['lgT_o'] = nc.dram_tensor('lgT_o', [NCLS, NQ], F32, kind="ExternalOutput").ap()
    do['bdy_o'] = nc.dram_tensor('bdy_o', [1, NQ], F32, kind="ExternalOutput").ap()

    with ExitStack() as ctx:
        tc = ctx.enter_context(tile.TileContext(nc))
        _emit(ctx, tc, nc, di, do)
    nc.compile()
    return nc


def _emit(ctx, tc, nc, di, do):
    const = ctx.enter_context(tc.tile_pool(name="const", bufs=1))
    pers = ctx.enter_context(tc.tile_pool(name="pers", bufs=1))
    wk = ctx.enter_context(tc.tile_pool(name="wk", bufs=2))      # stage-1 chunks
    stg = ctx.enter_context(tc.tile_pool(name="stg", bufs=3))    # G staging
    vpool = ctx.enter_context(tc.tile_pool(name="vpool", bufs=2))
    s2 = ctx.enter_context(tc.tile_pool(name="s2", bufs=2))
    sm = ctx.enter_context(tc.tile_pool(name="sm", bufs=4))
    ps = ctx.enter_context(tc.tile_pool(name="ps", bufs=8, space="PSUM"))
    dram = ctx.enter_context(tc.tile_pool(name="dram", bufs=1, space="DRAM"))

    def load(name, pool=const):
        t = pool.tile(list(di[name].shape), di[name].dtype, tag=name)
        nc.scalar.dma_start(out=t, in_=di[name])
        return t

    L1 = load('L1'); L2 = load('L2'); L3a = load('L3a'); L3b = load('L3b')
    L4 = load('L4'); L5 = load('L5'); L6 = load('L6'); L7 = load('L7')
    L8 = load('L8'); bias = load('bias'); ident = load('ident')

    def bcol(j, prt=128):
        return bias[:prt, j:j + 1]

    # persistent per-core tensors
    p = pers.tile([64, NQ], F32)      # pos-enc query vector (query half)
    QQ = pers.tile([128, NQ], F32)    # [Q' ; Q''] (query half)
    featT = load('featT', pers)
    xyzT = load('xyzT', pers)
    idx = load('idx', pers)
    G = dram.tile([N, GROW], F32)     # gather table
    Gb = G.bitcast(BF16)              # (N, 256) bf16 view

    # ---- stage 1: pointwise encoder, streamed in 512-column chunks ----
    # stacked layout: partitions = [half-A ch | half-B ch]
    for i in range(NC1):
        sl = slice(i * 512, (i + 1) * 512)
        ft = featT[:, sl]
        xy = xyzT[:, sl]

        pt = ps.tile([128, 512], F32, tag="ps")
        nc.tensor.matmul(pt, L1, ft, start=True, stop=True)
        hc = wk.tile([128, 512], F32, tag="h")
        nc.scalar.activation(hc, pt, AF.Relu, scale=bcol(0), bias=bcol(1))

        pt = ps.tile([128, 512], F32, tag="ps")
        nc.tensor.matmul(pt, L2, hc, start=True, stop=True)
        g0 = wk.tile([128, 512], F32, tag="g0")
        nc.scalar.activation(g0, pt, AF.Relu, scale=bcol(2), bias=bcol(3))

        pt = ps.tile([128, 512], F32, tag="ps")
        nc.tensor.matmul(pt, L3a, ft, start=True, stop=True)
        sc = wk.tile([128, 512], F32, tag="sc")
        nc.scalar.activation(sc, pt, AF.Identity, scale=1.0, bias=bcol(4))

        pt = ps.tile([128, 512], F32, tag="ps")
        nc.tensor.matmul(pt, L3b, ft, start=True, stop=True)
        sh = wk.tile([128, 512], F32, tag="sh")
        nc.scalar.activation(sh, pt, AF.Identity, scale=1.0, bias=bcol(5))

        gm = wk.tile([128, 512], F32, tag="gm")
        nc.vector.tensor_mul(gm, g0, sc)
        nc.vector.tensor_add(gm, gm, sh)

        pt = ps.tile([128, 512], F32, tag="ps")
        nc.tensor.matmul(pt, L4, xy, start=True, stop=True)
        vc = wk.tile([128, 512], F32, tag="vc")
        nc.scalar.activation(vc, pt, AF.Copy)
        nc.scalar.activation(p[:, sl], pt[0:64, :], AF.Identity, scale=1.0,
                             bias=bcol(6, 64))

        pt = ps.tile([128, 512], F32, tag="ps")
        nc.tensor.matmul(pt, L5, gm[0:64, :], start=True, stop=True)
        nc.scalar.activation(QQ[:, sl], pt, AF.Identity, scale=1.0, bias=bcol(7))

        pt = ps.tile([128, 512], F32, tag="ps")
        nc.tensor.matmul(pt[0:32, :], L6, gm[0:64, :], start=True, stop=True)
        bd1 = wk.tile([32, 512], F32, tag="bd1")
        nc.scalar.activation(bd1, pt[0:32, :], AF.Relu, scale=bcol(8, 32),
                             bias=bcol(9, 32))
        pt = ps.tile([128, 512], F32, tag="ps")
        nc.tensor.matmul(pt[0:1, :], L7, bd1, start=True, stop=True)
        bdc = stg.tile([1, 512], F32, tag="bdc")
        nc.scalar.activation(bdc, pt[0:1, :], AF.Identity, scale=1.0, bias=bcol(10, 1))
        nc.sync.dma_start(out=do['bdy_o'][:, sl], in_=bdc)

        # G table rows for these 512 points (each half)
        for j in range(4):
            m0 = i * 512 + j * 128
            csl = slice(j * 128, (j + 1) * 128)
            gT = ps.tile([128, 128], F32, tag="ps")
            nc.tensor.transpose(gT, gm[:, csl], ident)
            vT = ps.tile([128, 128], F32, tag="ps")
            nc.tensor.transpose(vT, vc[:, csl], ident)
            gs = stg.tile([128, 128], F32, tag="gs")
            nc.scalar.copy(gs, gT)
            gb = stg.tile([128, 128], BF16, tag="gb")
            nc.scalar.copy(gb, gT)
            vb = stg.tile([128, 128], BF16, tag="vb")
            nc.vector.tensor_copy(vb, vT)
            nc.sync.dma_start(out=G[m0:m0 + 128, 0:QK], in_=gs[:, 0:QK])
            nc.sync.dma_start(out=G[NQ + m0:NQ + m0 + 128, 0:QK], in_=gs[:, QK:128])
            nc.sync.dma_start(out=Gb[m0:m0 + 128, 128:192], in_=vb[:, 0:QK])
            nc.sync.dma_start(out=Gb[NQ + m0:NQ + m0 + 128, 128:192], in_=vb[:, QK:128])
            nc.sync.dma_start(out=Gb[m0:m0 + 128, 192:256], in_=gb[:, 0:QK])
            nc.sync.dma_start(out=Gb[NQ + m0:NQ + m0 + 128, 192:256], in_=gb[:, QK:128])

    _bisect = int(os.environ.get('KBISECT', '0'))
    if _bisect == 1:
        return
    # ---- stage 2: gather + attention, 16 chunks of 256 queries ----
    for ci in range(CH):
        ixc = idx[:, ci * QC:(ci + 1) * QC]
        V = vpool.tile([128, IPC // 128, GROW], F32, tag="V")
        # SWDGE ring holds <2048 in-flight row-descriptors; split the chunk
        # gather into 1024-index sub-gathers over disjoint slot ranges.
        for sg in range(IPC // 1024):
            nc.gpsimd.dma_gather(
                out_ap=V[:, sg * 8:(sg + 1) * 8, :],
                in_ap=G,
                idxs_ap=idx[:, ci * QC + sg * 64:ci * QC + (sg + 1) * 64],
                num_idxs=1024,
                num_idxs_reg=1024,
                elem_size=GROW,
            )
        Vb = V.bitcast(BF16)
        if _bisect == 2:
            of = s2.tile([128, QK], F32, tag="of")
            nc.vector.tensor_copy(of, bass.AP(tensor=V.tensor, offset=V.offset,
                                              ap=[list(V.ap[0]), [1, QK]]))
            nc.sync.dma_start(out=do['of_o'][ci * QC:ci * QC + 128, :], in_=of)
            continue
        for t in range(TPC):
            nb = ci * TPC + t
            qsl = slice(nb * 128, (nb + 1) * 128)
            pT = ps.tile([128, QK], F32, tag="ps")
            nc.tensor.transpose(pT, p[:, qsl], ident[0:64, 0:64])
            qqT = ps.tile([128, 128], F32, tag="ps")
            nc.tensor.transpose(qqT, QQ[:, qsl], ident)
            pq = s2.tile([128, 192], BF16, tag="pq")   # [p | Q' | Q'']
            nc.scalar.copy(pq[:, 0:64], pT)
            nc.scalar.copy(pq[:, 64:192], qqT)

            # gathered views for this tile: slot = k*TPC + t
            val_bf = bass.AP(tensor=Vb.tensor, offset=Vb.offset + t * 256 + 192,
                             ap=[list(Vb.ap[0]), [TPC * 256, K], [1, QK]])
            v_bf = bass.AP(tensor=Vb.tensor, offset=Vb.offset + t * 256 + 128,
                           ap=[list(Vb.ap[0]), [TPC * 256, K], [1, QK]])
            val_f = bass.AP(tensor=V.tensor, offset=V.offset + t * 128,
                            ap=[list(V.ap[0]), [TPC * 128, K], [1, QK]])

            p_b = pq[:, 0:64].unsqueeze(1).broadcast_to((128, K, QK))
            qp_b = pq[:, 64:128].unsqueeze(1).broadcast_to((128, K, QK))
            qpp_b = pq[:, 128:192].unsqueeze(1).broadcast_to((128, K, QK))

            diff = s2.tile([128, K, QK], BF16, tag="diff")
            nc.vector.scalar_tensor_tensor(diff, v_bf, -1.0, p_b, OP.mult, OP.add)
            m1 = s2.tile([128, K, QK], BF16, tag="m1")
            nc.vector.scalar_tensor_tensor(m1, diff, 0.0, qpp_b, OP.max, OP.mult)
            r1 = sm.tile([128, K], F32, tag="r1")
            nc.vector.tensor_reduce(r1, m1, AX.X, OP.add)
            m2 = s2.tile([128, K, QK], BF16, tag="m2")
            nc.vector.tensor_mul(m2, val_bf, qp_b)
            r2 = sm.tile([128, K], F32, tag="r2")
            nc.vector.tensor_reduce(r2, m2, AX.X, OP.add)
            lg = sm.tile([128, K], F32, tag="lg")
            nc.vector.tensor_add(lg, r1, r2)
            nmax = sm.tile([128, 1], F32, tag="nmax")
            nc.vector.tensor_reduce(nmax, lg, AX.X, OP.max, negate=True)
            ex = sm.tile([128, K], F32, tag="ex")
            sume = sm.tile([128, 1], F32, tag="sume")
            nc.scalar.activation(ex, lg, AF.Exp, bias=nmax, scale=1.0, accum_out=sume)
            rs = sm.tile([128, 1], F32, tag="rs")
            nc.vector.reciprocal(rs, sume)
            aff = sm.tile([128, K], F32, tag="aff")
            nc.vector.tensor_scalar_mul(aff, ex, rs)
            nc.sync.dma_start(out=do['aff_o'][qsl, :], in_=aff)

            aff_b = aff.unsqueeze(2).broadcast_to((128, K, QK))
            m3 = s2.tile([128, K, QK], F32, tag="m3")
            nc.vector.tensor_mul(m3, val_f, aff_b)
            of = s2.tile([128, QK], F32, tag="of")
            nc.vector.tensor_reduce(of, m3.rearrange("p k c -> p c k"), AX.X, OP.add)
            nc.sync.dma_start(out=do['of_o'][qsl, :], in_=of)

            # classifier for this tile
            ofT = ps.tile([QK, 128], F32, tag="ps")
            nc.tensor.transpose(ofT, of, ident)
            ofs = s2.tile([QK, 128], F32, tag="ofs")
            nc.scalar.copy(ofs, ofT)
            lp = ps.tile([128, 128], F32, tag="ps")
            nc.tensor.matmul(lp[0:NCLS, :], L8, ofs, start=True, stop=True)
            lgt = s2.tile([NCLS, 128], F32, tag="lgt")
            nc.scalar.activation(lgt, lp[0:NCLS, :], AF.Identity, scale=1.0,
                                 bias=bcol(11, NCLS))
            nc.sync.dma_start(out=do['lgT_o'][:, qsl], in_=lgt)


def get_nc():
    if 'nc' not in _CACHE:
        _CACHE['nc'] = _build_nc()
    return _CACHE['nc']


def kernel(xyz, feat, k_idx, params, _trace=False):
    xyz = np.asarray(xyz, np.float32)
    feat = np.asarray(feat, np.float32)
    k_idx_np = np.asarray(k_idx, np.int32)
    in_maps = _host_prep(xyz, feat, k_idx_np, params)
    nc = get_nc()
    res = run_bass_kernel_spmd(nc, in_maps, core_ids=list(range(NCORES)),
                               trace=_trace)
    outs = res.results
    logits = np.zeros((B * N, NCLS), np.float32)
    affinity = np.zeros((B, N, K), np.float32)
    out_feat = np.zeros((B * N, QK), np.float32)
    bdy = np.zeros((B, 1, N), np.float32)
    for c in range(NCORES):
        b, h = c // 2, c % 2
        o = outs[c]
        rows = slice(b * N + h * NQ, b * N + (h + 1) * NQ)
        logits[rows] = o['lgT_o'].T
        out_feat[rows] = o['of_o']
        affinity[b, h * NQ:(h + 1) * NQ] = o['aff_o']
        bdy[b, 0, h * NQ:(h + 1) * NQ] = o['bdy_o'][0]
    if _trace:
        kernel.last_exec_time_ns = res.exec_time_ns
        kernel.last_results = res
    return logits, affinity, k_idx_np, out_feat, bdy
